# revision 11
# baseline (speedup 1.0000x reference)
"""Causal dilated 1D conv (KW=4, dilation=8) as shifted matmuls on 8 TRN2 cores.

out[b,o,t] = sum_{k,c} W[o, c*4+k] * x[b, c, t + k*8 - 24]

Sharding: data-parallel over batch (16 batches -> 2 per core). Each core runs
an identical program: all weights stationary in SBUF, x streamed in 512-wide
time blocks (+24 halo), and one accumulating PSUM group per (out-chunk,
time-block): 14 bf16 matmuls (c-chunks 0-2 x 4 taps, c-chunk 3 x taps 0-1)
plus ONE fp8-e4m3 DoubleRow matmul covering c-chunk 3's tap pair (2,3) as a
256-deep double-pumped contraction. PSUM is copied back via DVE and DMA'd out.

Precision budget: 1/8 of the contraction in e4m3 (x scaled by 1/2, W by 2 to
keep both operands in e4m3's normal range; the product is scale-neutral in
fp32 PSUM) measures 1.47e-2 max rel err vs the 2e-2 gate; the bf16 part adds
~2e-3 in quadrature. A second fp8 pair would be sqrt(2)*1.47e-2 = 2.08e-2 —
over the gate — so f=1/8 is the precision-optimal operating point.
DoubleRow replaces 2x216ns bf16 matmuls with one ~250ns matmul, saving
~200ns per group (~25us total).

bf16 weights enable FWL (fast weight load) so LDWEIGHTS hides behind the
matmul stream via the PE background weight buffer; steady-state MM issue gap
is the 216ns floor (512 cols @ 2.4GHz + NX dispatch), ~92% MFU. Startup:
~40 dummy matmuls on scratch SBUF warm the HAM clock gate (1.2->2.4GHz
takes ~3.4us of sustained PE activity) while the bootstrap DMAs land; first
time-block x tiles are interleaved with weights in first-consumption order.
The final output tile is computed in two half-width PSUM groups so its
drain+store overlaps the last matmuls. Measured ~433us total vs a ~429us
floor (prologue 5.7us + HAM-gated start ~10.5us + 419us stream + tail).
"""

import numpy as np

B = 16
C_IN = 512
C_OUT = 512
T = 8192
KW = 4
DIL = 8
PAD = (KW - 1) * DIL  # 24

N_CORES = 8
B_PER = B // N_CORES  # 2
P = 128
TBLK = 512
NT = T // TBLK        # 16
NCC = C_IN // P       # 4
NOC = C_OUT // P      # 4

# fp8 DoubleRow covers (cc=PCC, k in PKS) as one 256-deep contraction
PCC = 3
PKS = (2, 3)
SX = 0.5   # x is quantized to e4m3 at 0.5x scale
SW = 2.0   # W at 2x scale (product is scale-neutral)

_cache = {}


def _build():
    import concourse.tile as tile
    from concourse import bacc, mybir

    nc = bacc.Bacc("TRN2", target_bir_lowering=False, debug=False,
                   num_devices=N_CORES)
    x = nc.dram_tensor("x", [B_PER, C_IN, T + PAD], mybir.dt.bfloat16,
                       kind="ExternalInput").ap()
    # bf16 weights pre-arranged on host as [cc, tap, c=128, o=512]
    wt = nc.dram_tensor("wt", [NCC, KW, P, C_OUT], mybir.dt.bfloat16,
                        kind="ExternalInput").ap()
    # fp8 tap-pair operands: x8[b,c,j,t] = e4m3(SX*xpad[b,384+c,t+8*(2+j)])
    x8 = nc.dram_tensor("x8", [B_PER, P, 2, T], mybir.dt.float8e4,
                        kind="ExternalInput").ap()
    # w8[c,j,o] = e4m3(SW*W[o, 384+c, 2+j])
    w8 = nc.dram_tensor("w8", [P, 2, C_OUT], mybir.dt.float8e4,
                        kind="ExternalInput").ap()
    out = nc.dram_tensor("out", [B_PER, C_OUT, T], mybir.dt.float32,
                         kind="ExternalOutput").ap()
    f32 = mybir.dt.float32
    bf16 = mybir.dt.bfloat16
    fp8 = mybir.dt.float8e4
    DR = mybir.MatmulPerfMode.DoubleRow

    # bf16 (cc, k) list: everything except the fp8-covered pair
    cks = [(cc, k) for cc in range(NCC) for k in range(KW)
           if not (cc == PCC and k in PKS)]
    n_acc = len(cks)  # 14

    with tile.TileContext(nc) as tc:
        with tc.tile_pool(name="wpool", bufs=1) as wpool, \
             tc.tile_pool(name="xpool", bufs=8) as xpool, \
             tc.tile_pool(name="opool", bufs=8) as opool, \
             tc.tile_pool(name="pspool", bufs=8, space="PSUM") as pspool:

            def load_xt(b, tb):
                xts = []
                for cc in range(NCC):
                    xt = xpool.tile([P, TBLK + PAD], bf16,
                                    name=f"xt{cc}", tag=f"xt{cc}")
                    nc.sync.dma_start(
                        xt[:],
                        x[b, cc * P:(cc + 1) * P,
                          tb * TBLK: tb * TBLK + TBLK + PAD])
                    xts.append(xt)
                return xts

            def load_xt8(b, tb):
                xt8 = xpool.tile([P, 2, TBLK], fp8, name="xt8", tag="xt8")
                nc.sync.dma_start(
                    xt8[:], x8[b, :, :, tb * TBLK:(tb + 1) * TBLK])
                return xt8

            # Interleave the first time-block's x tiles with their matching
            # per-cc weight tiles in first-group consumption order, so the PE
            # starts as soon as xt0+w00 land and is fed continuously.
            first_xts = []
            wtiles = {}
            for cc in range(NCC):
                xt = xpool.tile([P, TBLK + PAD], bf16,
                                name=f"xt{cc}", tag=f"xt{cc}")
                if cc == 0:
                    # split the two critical-path transfers (first matmul
                    # needs xt0 + w00) so they move on parallel DMA queues
                    h = (TBLK + PAD) // 2
                    nc.sync.dma_start(xt[:, 0:h], x[0, 0:P, 0:h])
                    nc.sync.dma_start(xt[:, h:TBLK + PAD],
                                      x[0, 0:P, h:TBLK + PAD])
                else:
                    nc.sync.dma_start(xt[:], x[0, cc * P:(cc + 1) * P,
                                               0:TBLK + PAD])
                first_xts.append(xt)
                for k in range(KW):
                    if cc == PCC and k in PKS:
                        continue
                    wtile = wpool.tile([P, C_OUT], bf16, name=f"w_{cc}_{k}",
                                       tag=f"w_{cc}_{k}")
                    if cc == 0 and k == 0:
                        nc.sync.dma_start(wtile[:, 0:P], wt[0, 0, :, 0:P])
                        nc.sync.dma_start(wtile[:, P:C_OUT],
                                          wt[0, 0, :, P:C_OUT])
                    else:
                        nc.sync.dma_start(wtile[:], wt[cc, k])
                    wtiles[(cc, k)] = wtile
            w8t = wpool.tile([P, 2, C_OUT], fp8, name="w8", tag="w8")
            nc.sync.dma_start(w8t[:], w8[:])
            first_xt8 = load_xt8(0, 0)

            # Bootstrap block: emit MMs in weight-DMA-arrival order, fanning
            # each arriving weight across the 4 oc PSUM banks, so the in-order
            # PE stream is never head-of-line blocked on a later weight tile.
            pss0 = [pspool.tile([P, TBLK], f32, name="ps", tag="ps")
                    for _ in range(NOC)]

            # HAM pre-warm: ~40 dummy matmuls on scratch SBUF while the PE
            # would otherwise idle waiting for the bootstrap DMAs (~4.4us).
            # ~3.4us of sustained PE activity trips the HAM SHORT window, so
            # the real matmul stream starts at 2.4GHz instead of paying ~5us
            # of cold-clock. Results land in pss0[0] and are discarded: the
            # first real matmul runs with start=True, which clears the bank.
            scratch = wpool.tile([P, P], bf16, name="warm", tag="warm")
            nc.vector.memset(scratch[:], 0)
            for _ in range(36):
                nc.tensor.matmul(pss0[0][:, 0:P], scratch[:], scratch[:],
                                 start=True, stop=True)
            for ci, (cc, k) in enumerate(cks):
                for oc in range(NOC):
                    nc.tensor.matmul(
                        pss0[oc][:],
                        wtiles[(cc, k)][:, oc * P:(oc + 1) * P],
                        first_xts[cc][:, k * DIL: k * DIL + TBLK],
                        start=(ci == 0),
                        stop=False,
                    )
            for oc in range(NOC):
                nc.tensor.matmul(
                    pss0[oc][:],
                    w8t[:, :, oc * P:(oc + 1) * P],
                    first_xt8[:],
                    start=False,
                    stop=True,
                    perf_mode=DR,
                )
            for oc in range(NOC):
                ot = opool.tile([P, TBLK], f32, name="ot", tag="ot")
                nc.vector.tensor_copy(ot[:], pss0[oc][:])
                nc.sync.dma_start(out[0, oc * P:(oc + 1) * P, 0:TBLK], ot[:])

            def emit_group(b, tb, oc, xts, xt8, c0, cw):
                # one PSUM accumulation over output cols [c0, c0+cw) of the
                # (b, tb, oc) tile: 14 bf16 MMs + 1 fp8 DoubleRow MM
                ps = pspool.tile([P, cw], f32, name="ps", tag="ps")
                for ci, (cc, k) in enumerate(cks):
                    nc.tensor.matmul(
                        ps[:],
                        wtiles[(cc, k)][:, oc * P:(oc + 1) * P],
                        xts[cc][:, k * DIL + c0: k * DIL + c0 + cw],
                        start=(ci == 0),
                        stop=False,
                    )
                nc.tensor.matmul(
                    ps[:],
                    w8t[:, :, oc * P:(oc + 1) * P],
                    xt8[:, :, c0:c0 + cw],
                    start=False,
                    stop=True,
                    perf_mode=DR,
                )
                ot = opool.tile([P, cw], f32, name="ot", tag="ot")
                nc.vector.tensor_copy(ot[:], ps[:])
                nc.sync.dma_start(
                    out[b, oc * P:(oc + 1) * P,
                        tb * TBLK + c0: tb * TBLK + c0 + cw],
                    ot[:])

            for b in range(B_PER):
                for tb in range(NT):
                    if b == 0 and tb == 0:
                        continue
                    xts = load_xt(b, tb)
                    xt8 = load_xt8(b, tb)
                    last = (b == B_PER - 1 and tb == NT - 1)
                    for oc in range(NOC):
                        if last and oc == NOC - 1:
                            # split the final group in half so the first
                            # half's PSUM drain + store overlaps the second
                            # half's matmuls, shrinking the exposed tail
                            emit_group(b, tb, oc, xts, xt8, 0, TBLK // 2)
                            emit_group(b, tb, oc, xts, xt8, TBLK // 2,
                                       TBLK // 2)
                        else:
                            emit_group(b, tb, oc, xts, xt8, 0, TBLK)

    nc.compile()
    return nc


def _get_nc():
    if "nc" not in _cache:
        _cache["nc"] = _build()
    return _cache["nc"]


def _make_in_maps(x, W):
    import ml_dtypes
    bf16 = ml_dtypes.bfloat16
    e4m3 = ml_dtypes.float8_e4m3
    xpad = np.pad(np.ascontiguousarray(x, dtype=np.float32),
                  ((0, 0), (0, 0), (PAD, 0)))
    w = np.ascontiguousarray(W, dtype=np.float32).reshape(C_OUT, C_IN, KW)
    # wt[cc, k, c, o] = W[o, (cc*128+c)*KW + k]
    wt = np.transpose(w.reshape(C_OUT, NCC, P, KW),
                      (1, 3, 2, 0)).astype(bf16).copy()
    # fp8 pair operands (cc=PCC, taps PKS), product-neutral scales
    base = xpad[:, PCC * P:(PCC + 1) * P, :]            # [B, 128, T+PAD]
    x8 = np.stack([base[:, :, PKS[0] * DIL: PKS[0] * DIL + T],
                   base[:, :, PKS[1] * DIL: PKS[1] * DIL + T]],
                  axis=2)                               # [B, 128, 2, T]
    x8 = (x8 * SX).astype(e4m3)
    w8 = np.transpose(w[:, PCC * P:(PCC + 1) * P, PKS[0]:PKS[1] + 1],
                      (1, 2, 0))                        # [128, 2, 512]
    w8 = (w8 * SW).astype(e4m3).copy()
    xpad16 = xpad.astype(bf16)
    return [{"x": np.ascontiguousarray(xpad16[i * B_PER:(i + 1) * B_PER]),
             "x8": np.ascontiguousarray(x8[i * B_PER:(i + 1) * B_PER]),
             "wt": wt, "w8": w8} for i in range(N_CORES)]


def kernel(x, W):
    from concourse.bass_utils import run_bass_kernel_spmd

    nc = _get_nc()
    in_maps = _make_in_maps(x, W)
    res = run_bass_kernel_spmd(nc, in_maps, list(range(N_CORES)))
    return np.concatenate([r["out"] for r in res.results], axis=0)


# revision 13
# speedup vs baseline: 1.0038x; 1.0038x over previous
"""Causal dilated 1D conv (KW=4, dilation=8) as shifted matmuls on 8 TRN2 cores.

out[b,o,t] = sum_{k,c} W[o, c*4+k] * x[b, c, t + k*8 - 24]

Sharding: data-parallel over batch (16 batches -> 2 per core). Each core runs
an identical program: all weights stationary in SBUF, x streamed in 512-wide
time blocks (+24 halo), and one accumulating PSUM group per (out-chunk,
time-block): 14 bf16 matmuls (c-chunks 0-2 x 4 taps, c-chunk 3 x taps 0-1)
plus ONE fp8-e4m3 DoubleRow matmul covering c-chunk 3's tap pair (2,3) as a
256-deep double-pumped contraction. PSUM is copied back via DVE and DMA'd out.

Precision budget: 1/8 of the contraction in e4m3 (x scaled by 1/2, W by 2 to
keep both operands in e4m3's normal range; the product is scale-neutral in
fp32 PSUM) measures 1.47e-2 max rel err vs the 2e-2 gate; the bf16 part adds
~2e-3 in quadrature. A second fp8 pair would be sqrt(2)*1.47e-2 = 2.08e-2 —
over the gate — so f=1/8 is the precision-optimal operating point.
DoubleRow replaces 2x216ns bf16 matmuls with one ~250ns matmul, saving
~200ns per group (~25us total).

bf16 weights enable FWL (fast weight load) so LDWEIGHTS hides behind the
matmul stream via the PE background weight buffer; steady-state MM issue gap
is the 216ns floor (512 cols @ 2.4GHz + NX dispatch), ~92% MFU. Startup:
~40 dummy matmuls on scratch SBUF warm the HAM clock gate (1.2->2.4GHz
takes ~3.4us of sustained PE activity) while the bootstrap DMAs land; first
time-block x tiles are interleaved with weights in first-consumption order.
The final output tile is computed in two half-width PSUM groups so its
drain+store overlaps the last matmuls. Measured ~433us total vs a ~429us
floor (prologue 5.7us + HAM-gated start ~10.5us + 419us stream + tail).
"""

import numpy as np

B = 16
C_IN = 512
C_OUT = 512
T = 8192
KW = 4
DIL = 8
PAD = (KW - 1) * DIL  # 24

N_CORES = 8
B_PER = B // N_CORES  # 2
P = 128
TBLK = 512
NT = T // TBLK        # 16
NCC = C_IN // P       # 4
NOC = C_OUT // P      # 4

# fp8 DoubleRow covers (cc=PCC, k in PKS) as one 256-deep contraction
PCC = 3
PKS = (2, 3)
SX = 0.5   # x is quantized to e4m3 at 0.5x scale
SW = 2.0   # W at 2x scale (product is scale-neutral)

_cache = {}


def _build():
    import concourse.tile as tile
    from concourse import bacc, mybir

    nc = bacc.Bacc("TRN2", target_bir_lowering=False, debug=False,
                   num_devices=N_CORES)
    x = nc.dram_tensor("x", [B_PER, C_IN, T + PAD], mybir.dt.bfloat16,
                       kind="ExternalInput").ap()
    # bf16 weights pre-arranged on host as [cc, tap, c=128, o=512]
    wt = nc.dram_tensor("wt", [NCC, KW, P, C_OUT], mybir.dt.bfloat16,
                        kind="ExternalInput").ap()
    # fp8 tap-pair operands: x8[b,c,j,t] = e4m3(SX*xpad[b,384+c,t+8*(2+j)])
    x8 = nc.dram_tensor("x8", [B_PER, P, 2, T], mybir.dt.float8e4,
                        kind="ExternalInput").ap()
    # w8[c,j,o] = e4m3(SW*W[o, 384+c, 2+j])
    w8 = nc.dram_tensor("w8", [P, 2, C_OUT], mybir.dt.float8e4,
                        kind="ExternalInput").ap()
    out = nc.dram_tensor("out", [B_PER, C_OUT, T], mybir.dt.float32,
                         kind="ExternalOutput").ap()
    f32 = mybir.dt.float32
    bf16 = mybir.dt.bfloat16
    fp8 = mybir.dt.float8e4
    DR = mybir.MatmulPerfMode.DoubleRow

    # bf16 (cc, k) list: everything except the fp8-covered pair
    cks = [(cc, k) for cc in range(NCC) for k in range(KW)
           if not (cc == PCC and k in PKS)]
    n_acc = len(cks)  # 14

    with tile.TileContext(nc) as tc:
        with tc.tile_pool(name="wpool", bufs=1) as wpool, \
             tc.tile_pool(name="xpool", bufs=8) as xpool, \
             tc.tile_pool(name="opool", bufs=8) as opool, \
             tc.tile_pool(name="pspool", bufs=8, space="PSUM") as pspool:

            def load_xt(b, tb):
                xts = []
                for cc in range(NCC):
                    xt = xpool.tile([P, TBLK + PAD], bf16,
                                    name=f"xt{cc}", tag=f"xt{cc}")
                    nc.sync.dma_start(
                        xt[:],
                        x[b, cc * P:(cc + 1) * P,
                          tb * TBLK: tb * TBLK + TBLK + PAD])
                    xts.append(xt)
                return xts

            def load_xt8(b, tb):
                xt8 = xpool.tile([P, 2, TBLK], fp8, name="xt8", tag="xt8")
                nc.sync.dma_start(
                    xt8[:], x8[b, :, :, tb * TBLK:(tb + 1) * TBLK])
                return xt8

            # Interleave the first time-block's x tiles with their matching
            # per-cc weight tiles in first-group consumption order, so the PE
            # starts as soon as xt0+w00 land and is fed continuously.
            first_xts = []
            wtiles = {}
            for cc in range(NCC):
                xt = xpool.tile([P, TBLK + PAD], bf16,
                                name=f"xt{cc}", tag=f"xt{cc}")
                nc.sync.dma_start(xt[:], x[0, cc * P:(cc + 1) * P,
                                           0:TBLK + PAD])
                first_xts.append(xt)
                for k in range(KW):
                    if cc == PCC and k in PKS:
                        continue
                    wtile = wpool.tile([P, C_OUT], bf16, name=f"w_{cc}_{k}",
                                       tag=f"w_{cc}_{k}")
                    nc.sync.dma_start(wtile[:], wt[cc, k])
                    wtiles[(cc, k)] = wtile
            w8t = wpool.tile([P, 2, C_OUT], fp8, name="w8", tag="w8")
            nc.sync.dma_start(w8t[:], w8[:])
            first_xt8 = load_xt8(0, 0)

            # Bootstrap block: emit MMs in weight-DMA-arrival order, fanning
            # each arriving weight across the 4 oc PSUM banks, so the in-order
            # PE stream is never head-of-line blocked on a later weight tile.
            pss0 = [pspool.tile([P, TBLK], f32, name="ps", tag="ps")
                    for _ in range(NOC)]

            # HAM pre-warm: ~40 dummy matmuls on scratch SBUF while the PE
            # would otherwise idle waiting for the bootstrap DMAs (~4.4us).
            # ~3.4us of sustained PE activity trips the HAM SHORT window, so
            # the real matmul stream starts at 2.4GHz instead of paying ~5us
            # of cold-clock. Results land in pss0[0] and are discarded: the
            # first real matmul runs with start=True, which clears the bank.
            scratch = wpool.tile([P, P], bf16, name="warm", tag="warm")
            nc.vector.memset(scratch[:], 0)
            for _ in range(40):
                nc.tensor.matmul(pss0[0][:, 0:P], scratch[:], scratch[:],
                                 start=True, stop=True)
            for ci, (cc, k) in enumerate(cks):
                for oc in range(NOC):
                    nc.tensor.matmul(
                        pss0[oc][:],
                        wtiles[(cc, k)][:, oc * P:(oc + 1) * P],
                        first_xts[cc][:, k * DIL: k * DIL + TBLK],
                        start=(ci == 0),
                        stop=False,
                    )
            for oc in range(NOC):
                nc.tensor.matmul(
                    pss0[oc][:],
                    w8t[:, :, oc * P:(oc + 1) * P],
                    first_xt8[:],
                    start=False,
                    stop=True,
                    perf_mode=DR,
                )
            for oc in range(NOC):
                ot = opool.tile([P, TBLK], f32, name="ot", tag="ot")
                nc.vector.tensor_copy(ot[:], pss0[oc][:])
                nc.sync.dma_start(out[0, oc * P:(oc + 1) * P, 0:TBLK], ot[:])

            def emit_group(b, tb, oc, xts, xt8, c0, cw):
                # one PSUM accumulation over output cols [c0, c0+cw) of the
                # (b, tb, oc) tile: 14 bf16 MMs + 1 fp8 DoubleRow MM
                ps = pspool.tile([P, cw], f32, name="ps", tag="ps")
                for ci, (cc, k) in enumerate(cks):
                    nc.tensor.matmul(
                        ps[:],
                        wtiles[(cc, k)][:, oc * P:(oc + 1) * P],
                        xts[cc][:, k * DIL + c0: k * DIL + c0 + cw],
                        start=(ci == 0),
                        stop=False,
                    )
                nc.tensor.matmul(
                    ps[:],
                    w8t[:, :, oc * P:(oc + 1) * P],
                    xt8[:, :, c0:c0 + cw],
                    start=False,
                    stop=True,
                    perf_mode=DR,
                )
                ot = opool.tile([P, cw], f32, name="ot", tag="ot")
                nc.vector.tensor_copy(ot[:], ps[:])
                nc.sync.dma_start(
                    out[b, oc * P:(oc + 1) * P,
                        tb * TBLK + c0: tb * TBLK + c0 + cw],
                    ot[:])

            for b in range(B_PER):
                for tb in range(NT):
                    if b == 0 and tb == 0:
                        continue
                    xts = load_xt(b, tb)
                    xt8 = load_xt8(b, tb)
                    last = (b == B_PER - 1 and tb == NT - 1)
                    for oc in range(NOC):
                        if last and oc == NOC - 1:
                            # split the final group in half so the first
                            # half's PSUM drain + store overlaps the second
                            # half's matmuls, shrinking the exposed tail
                            emit_group(b, tb, oc, xts, xt8, 0, TBLK // 2)
                            emit_group(b, tb, oc, xts, xt8, TBLK // 2,
                                       TBLK // 2)
                        else:
                            emit_group(b, tb, oc, xts, xt8, 0, TBLK)

    nc.compile()
    return nc


def _get_nc():
    if "nc" not in _cache:
        _cache["nc"] = _build()
    return _cache["nc"]


def _make_in_maps(x, W):
    import ml_dtypes
    bf16 = ml_dtypes.bfloat16
    e4m3 = ml_dtypes.float8_e4m3
    xpad = np.pad(np.ascontiguousarray(x, dtype=np.float32),
                  ((0, 0), (0, 0), (PAD, 0)))
    w = np.ascontiguousarray(W, dtype=np.float32).reshape(C_OUT, C_IN, KW)
    # wt[cc, k, c, o] = W[o, (cc*128+c)*KW + k]
    wt = np.transpose(w.reshape(C_OUT, NCC, P, KW),
                      (1, 3, 2, 0)).astype(bf16).copy()
    # fp8 pair operands (cc=PCC, taps PKS), product-neutral scales
    base = xpad[:, PCC * P:(PCC + 1) * P, :]            # [B, 128, T+PAD]
    x8 = np.stack([base[:, :, PKS[0] * DIL: PKS[0] * DIL + T],
                   base[:, :, PKS[1] * DIL: PKS[1] * DIL + T]],
                  axis=2)                               # [B, 128, 2, T]
    x8 = (x8 * SX).astype(e4m3)
    w8 = np.transpose(w[:, PCC * P:(PCC + 1) * P, PKS[0]:PKS[1] + 1],
                      (1, 2, 0))                        # [128, 2, 512]
    w8 = (w8 * SW).astype(e4m3).copy()
    xpad16 = xpad.astype(bf16)
    return [{"x": np.ascontiguousarray(xpad16[i * B_PER:(i + 1) * B_PER]),
             "x8": np.ascontiguousarray(x8[i * B_PER:(i + 1) * B_PER]),
             "wt": wt, "w8": w8} for i in range(N_CORES)]


def kernel(x, W):
    from concourse.bass_utils import run_bass_kernel_spmd

    nc = _get_nc()
    in_maps = _make_in_maps(x, W)
    res = run_bass_kernel_spmd(nc, in_maps, list(range(N_CORES)))
    return np.concatenate([r["out"] for r in res.results], axis=0)
